# revision 61
# baseline (speedup 1.0000x reference)
# Binary linear: y[b,s,o] = sum_i x[b,s,i] * sign(W)[o,i]
#
# Strategy (8 NeuronCores, data-parallel over tokens):
#   - Host: flatten x to [32768, 768] and shard 8 x [4096, 768]. Per core,
#     pack x per 512-token group with the contraction dim on SBUF
#     partitions, p-major so every DMA lands with multi-KB contiguous
#     partition rows (small rows halve the DMA queues' effective rate).
#     Contraction blocks k0/k1 are quantized to fp8 e4m3, k2..k5 stay bf16.
#     Weights are sign(W) (exactly +-1): fp8 for k0/k1, bf16 for k2..k5.
#   - Device (per core): out[o-block, token] layout. Per (group, out-slab):
#     four bf16 matmuls (k2..k5, N=512) plus ONE fp8 DoubleRow matmul that
#     contracts k0+k1 together at ~1.44x the bf16 rate. Out-slabs run in
#     PAIRS with the k-loop interleaved between the two PSUM banks: a
#     single bank caps the accumulate stream at ~2.0GHz, alternating banks
#     sustains the full ~2.4GHz PE rate. Evictions are DVE f32->bf16 casts
#     (kept off the scalar engine, whose DMA issues would delay them and
#     stall PSUM recycling); y stores are linear 128KB DMAs balanced across
#     the two hardware DMA queues. A short PE warmup covers the ~3us DMA
#     launch+ramp latency.
#   - Accuracy: only x carries rounding error (w is exact): fp8 on 2/6 of
#     the contraction + bf16 elsewhere + bf16 y => rel err ~1.55e-2
#     (measured), within the 2e-2 gate with margin.
#   - Host: unpack [os][g][128, 512] -> [4, 8192, 768] f32.

import numpy as np

N_CORES = 8
B, S, D_IN, D_OUT = 4, 8192, 768, 768
T_TOTAL = B * S              # 32768 tokens
T_CORE = T_TOTAL // N_CORES  # 4096 tokens per core
P = 128
KB = D_IN // P               # 6 contraction blocks (k0/k1 fp8, k2-5 bf16)
OS = D_OUT // P              # 6 out-feature slabs
TG = 512                     # tokens per group (one PSUM bank of f32)
G = T_CORE // TG             # 8 groups per core
N_WARMUP = 5

_cache = {}


def _build():
    import concourse.bacc as bacc
    import concourse.mybir as mybir
    import concourse.tile as tile

    f32 = mybir.dt.float32
    bf16 = mybir.dt.bfloat16
    fp8 = mybir.dt.float8e4
    DR = mybir.MatmulPerfMode.DoubleRowSwInterleave

    nc = bacc.Bacc(
        "TRN2",
        target_bir_lowering=False,
        debug=False,
        num_devices=N_CORES,
    )

    # group 0 (1/8 of tokens) is computed FULLY in fp8 (3 DoubleRows instead
    # of 1 DR + 4 bf16 matmuls): saves ~2.5us of PE stream and shrinks the
    # critical head bytes; total rel err 1.73e-2 (measured) vs the 2e-2 gate
    x8P = nc.dram_tensor("x8P", [G, P, 2, TG], fp8, kind="ExternalInput")
    x8g0P = nc.dram_tensor("x8g0P", [2, P, 2, TG], fp8, kind="ExternalInput")
    xBP = nc.dram_tensor("xBP", [G, P, KB - 2, TG], bf16, kind="ExternalInput")
    w8P = nc.dram_tensor("w8P", [3, P, OS, 2, P], fp8, kind="ExternalInput")
    wBP = nc.dram_tensor("wBP", [OS, P, KB - 2, P], bf16, kind="ExternalInput")
    yP = nc.dram_tensor("yP", [OS, G, P, TG], bf16, kind="ExternalOutput")

    with tile.TileContext(nc) as tc:
        with (
            tc.tile_pool(name="wpool", bufs=1) as w_pool,
            tc.tile_pool(name="xpool", bufs=1) as x_pool,
            tc.tile_pool(name="ypool", bufs=14) as y_pool,
            tc.tile_pool(name="psum", bufs=7, space="PSUM") as psum_pool,
        ):
            # --- PE warmup: dummy matmuls on zeroed scratch so the PE clock
            # has ramped by the time the first real operands land. ---
            wu = x_pool.tile([P, P + TG], bf16, tag="wu", name="wu", bufs=1)
            nc.gpsimd.memset(wu[:], 0.0)
            wups = psum_pool.tile([P, TG], f32, tag="wups", name="wups", bufs=1)
            for _ in range(N_WARMUP):
                nc.tensor.matmul(
                    wups[:], wu[:, :P], wu[:, P:],
                    start=True, stop=True, skip_group_check=True,
                )
            wu_out = x_pool.tile([P, TG], bf16, tag="wuo", name="wuo", bufs=1)
            nc.vector.tensor_copy(wu_out[:], wups[:])

            # --- loads, interleaved across the two HW queues in need order
            # (group 0 consumes only fp8 operands; for g>=1 the k-order is
            # bf16 k2..k5 first, fp8 DR last, so the fp8 operands may
            # arrive later than the bf16 ones)
            w8k = [
                w_pool.tile([P, OS, 2, P], fp8, tag=f"w8_{kp}", name=f"w8_{kp}")
                for kp in range(3)
            ]
            x8g0kp = [
                x_pool.tile([P, 2, TG], fp8, tag=f"x8g0_{kp}", name=f"x8g0_{kp}")
                for kp in range(2)
            ]
            wB = [None] * OS

            def wB_load(os_, eng):
                t = w_pool.tile([P, KB - 2, P], bf16, tag=f"wB{os_}", name=f"wB{os_}")
                eng.dma_start(t[:], wBP[os_])
                wB[os_] = t

            x8 = [None] * G

            def x8_load(g, eng):
                t = x_pool.tile([P, 2, TG], fp8, tag=f"x8_{g}", name=f"x8_{g}")
                eng.dma_start(t[:], x8P[g])
                x8[g] = t

            xB = [None] * G
            xB1h = [None, None]

            def xB1_load(h, eng):
                # group 1 bf16 x as two k-pair tiles (2KB contiguous rows)
                # so both queues fill it in parallel just ahead of the PE
                t = x_pool.tile([P, 2, TG], bf16, tag=f"xB1_{h}", name=f"xB1_{h}")
                eng.dma_start(t[:], xBP[1, :, 2 * h : 2 * h + 2, :])
                xB1h[h] = t

            def xB_load(g, eng):
                t = x_pool.tile([P, KB - 2, TG], bf16, tag=f"xB{g}", name=f"xB{g}")
                eng.dma_start(t[:], xBP[g])
                xB[g] = t

            x8_load(0, nc.sync)
            nc.scalar.dma_start(w8k[0][:], w8P[0])
            nc.sync.dma_start(w8k[1][:], w8P[1])
            nc.scalar.dma_start(x8g0kp[0][:], x8g0P[0])
            nc.sync.dma_start(x8g0kp[1][:], x8g0P[1])
            nc.scalar.dma_start(w8k[2][:], w8P[2])
            wB_load(0, nc.sync)
            wB_load(1, nc.scalar)
            x8_load(1, nc.scalar)
            xB1_load(0, nc.sync)
            xB1_load(1, nc.scalar)
            wB_load(2, nc.sync)
            wB_load(3, nc.scalar)
            wB_load(4, nc.scalar)
            wB_load(5, nc.scalar)
            xB_load(3, nc.sync)
            xB_load(2, nc.scalar)
            x8_load(2, nc.scalar)
            x8_load(3, nc.scalar)
            xB_load(5, nc.sync)
            xB_load(4, nc.scalar)
            x8_load(4, nc.scalar)
            x8_load(5, nc.scalar)
            xB_load(6, nc.sync)
            x8_load(6, nc.scalar)
            x8_load(7, nc.scalar)
            xB_load(7, nc.sync)

            def rhsB(g, k):
                if g == 1:
                    return xB1h[(k - 2) // 2][:, (k - 2) % 2, :]
                return xB[g][:, k - 2, :]

            # --- main loop: out-slab pairs, k-loop interleaved across the
            # pair's two PSUM banks to sustain the full PE rate; bf16 k2..k5
            # first, then one fp8 DoubleRow matmul contracting k0+k1 ---
            ecnt = 0
            for g in range(G):
                for osp in range(OS // 2):
                    os_a, os_b = 2 * osp, 2 * osp + 1
                    final = g == G - 1 and osp == OS // 2 - 1
                    ps_a = psum_pool.tile([P, TG], f32, tag="ps", name=f"ps{g}_{os_a}")
                    ps_b = psum_pool.tile([P, TG], f32, tag="ps", name=f"ps{g}_{os_b}")
                    if g == 0:
                        # group 0: fully fp8 - three DoubleRow pairs
                        for kp in range(3):
                            r8 = x8[0][:] if kp == 0 else x8g0kp[kp - 1][:]
                            st, sp = kp == 0, kp == 2
                            nc.tensor.matmul(
                                ps_a[:], w8k[kp][:, os_a, :, :], r8,
                                start=st, stop=sp, perf_mode=DR,
                            )
                            nc.tensor.matmul(
                                ps_b[:], w8k[kp][:, os_b, :, :], r8,
                                start=st, stop=sp, perf_mode=DR,
                            )
                    else:
                        for k in range(2, KB):
                            st = k == 2
                            nc.tensor.matmul(
                                ps_a[:], wB[os_a][:, k - 2, :], rhsB(g, k),
                                start=st, stop=False, skip_group_check=final,
                            )
                            nc.tensor.matmul(
                                ps_b[:], wB[os_b][:, k - 2, :], rhsB(g, k),
                                start=st, stop=False, skip_group_check=final,
                            )
                    if final:
                        # tail pair: split the closing DR matmuls into
                        # column halves so the left half's eviction+store
                        # overlaps the right half's compute, and fan the
                        # four lanes over both copy engines and DMA queues
                        h = TG // 2
                        yt_a = y_pool.tile([P, TG], bf16, tag="y", name="yfa")
                        yt_b = y_pool.tile([P, TG], bf16, tag="y", name="yfb")
                        # all four half-DRs first (emitting evictions in
                        # between makes Tile serialize the later DRs behind
                        # the earlier reads), then the four evict+store
                        # lanes - the left lanes' sems only depend on the
                        # left DRs, so they overlap the right DRs' compute
                        for lo, hi in ((0, h), (h, TG)):
                            r8 = x8[g][:, :, lo:hi]
                            nc.tensor.matmul(
                                ps_a[:, lo:hi], w8k[0][:, os_a, :, :], r8,
                                start=False, stop=hi == TG, perf_mode=DR,
                                skip_group_check=True,
                            )
                            nc.tensor.matmul(
                                ps_b[:, lo:hi], w8k[0][:, os_b, :, :], r8,
                                start=False, stop=hi == TG, perf_mode=DR,
                                skip_group_check=True,
                            )
                        for lo, hi in ((0, h), (h, TG)):
                            nc.vector.tensor_copy(
                                yt_a[:, lo:hi], ps_a[:, lo:hi]
                            )
                            nc.scalar.copy(yt_b[:, lo:hi], ps_b[:, lo:hi])
                            nc.sync.dma_start(
                                yP[os_a, g, :, lo:hi], yt_a[:, lo:hi]
                            )
                            nc.scalar.dma_start(
                                yP[os_b, g, :, lo:hi], yt_b[:, lo:hi]
                            )
                        continue
                    if g != 0:
                        rhs8 = x8[g][:]
                        nc.tensor.matmul(
                            ps_a[:], w8k[0][:, os_a, :, :], rhs8,
                            start=False, stop=True, perf_mode=DR,
                        )
                        nc.tensor.matmul(
                            ps_b[:], w8k[0][:, os_b, :, :], rhs8,
                            start=False, stop=True, perf_mode=DR,
                        )
                    for os_, ps in ((os_a, ps_a), (os_b, ps_b)):
                        yt = y_pool.tile([P, TG], bf16, tag="y", name=f"y{g}_{os_}")
                        # all evictions on DVE: the scalar engine's DMA
                        # issues would delay them and stall PSUM recycling
                        nc.vector.tensor_copy(yt[:], ps[:])
                        # store queues: scalar while sync still streams x,
                        # alternating afterwards so neither queue backs up
                        # near the tail
                        if g <= 4:
                            q = nc.scalar
                        else:
                            q = nc.sync if os_ % 2 == 0 else nc.scalar
                        q.dma_start(yP[os_, g], yt[:])
                        ecnt += 1

    nc.compile()
    return nc


def _get_nc():
    if "nc" not in _cache:
        _cache["nc"] = _build()
    return _cache["nc"]


def _prep_inputs(x, weight):
    import ml_dtypes

    x = np.asarray(x, dtype=np.float32)
    w = np.asarray(weight, dtype=np.float32)
    x2 = x.reshape(N_CORES, T_CORE, D_IN)
    # x5[c, g, t, k, p] = x2[c, g*TG + t, k*P + p] -> packs [c, g, p, k, t]
    x5 = x2.reshape(N_CORES, G, TG, KB, P)
    x8Pack = np.ascontiguousarray(x5[:, :, :, :2, :].transpose(0, 1, 4, 3, 2)).astype(
        ml_dtypes.float8_e4m3fn
    )
    xBPack = np.ascontiguousarray(x5[:, :, :, 2:, :].transpose(0, 1, 4, 3, 2)).astype(
        ml_dtypes.bfloat16
    )
    # group-0 k2..k5 x in fp8 as two pair-major k-pair tiles [c, kp, p, 2, t]
    x8g0Pack = np.ascontiguousarray(
        x5[:, 0, :, 2:, :].transpose(0, 3, 2, 1)          # [c, p, k4, t]
        .reshape(N_CORES, P, 2, 2, TG).transpose(0, 2, 1, 3, 4)
    ).astype(ml_dtypes.float8_e4m3fn)
    # S4[os, o, k, p] = sign(W)[os*P + o, k*P + p]  (+-1/0 exact in both)
    S4 = np.sign(w).reshape(OS, P, KB, P)
    # SwInterleave weight layout, one pack per k-pair: per partition row the
    # pair's weights are interleaved pairwise with output columns reversed:
    # [A127, B127, A126, B126, ..., A0, B0]  (A=2kp, B=2kp+1)
    w8Pack = np.ascontiguousarray(
        np.stack(
            [
                S4[:, ::-1, 2 * kp : 2 * kp + 2, :]
                .transpose(3, 0, 1, 2)
                .reshape(P, OS, 2, P)
                for kp in range(3)
            ]
        )
    ).astype(ml_dtypes.float8_e4m3fn)
    wBPack = np.ascontiguousarray(S4[:, :, 2:, :].transpose(0, 3, 2, 1)).astype(
        ml_dtypes.bfloat16
    )
    return [
        {
            "x8P": x8Pack[c],
            "x8g0P": x8g0Pack[c],
            "xBP": xBPack[c],
            "w8P": w8Pack,
            "wBP": wBPack,
        }
        for c in range(N_CORES)
    ]


def _unpack_output(res):
    # yP [OS, G, P(o), TG(t)] -> y_core [T_CORE, D_OUT]
    outs = []
    for r in res.results:
        yp = np.asarray(r["yP"]).astype(np.float32)
        outs.append(yp.transpose(1, 3, 0, 2).reshape(T_CORE, D_OUT))
    return np.concatenate(outs, axis=0).reshape(B, S, D_OUT)


def _install_axon_ntff_hook():
    """The agent image's `antenv` lacks `axon_hooks`; register an equivalent
    module backed by direct ctypes calls into libaxon_pjrt.so so that
    run_bass_kernel_spmd(trace=True) can capture NTFF profiles under axon."""
    import sys

    if "antenv.axon_hooks" in sys.modules:
        return
    import contextlib
    import ctypes
    import types

    so_path = "/opt/axon/libaxon_pjrt.so"
    try:
        lib = ctypes.CDLL(so_path)
    except OSError:
        return
    if not hasattr(lib, "axon_start_nrt_profile"):
        return
    lib.axon_start_nrt_profile.argtypes = [
        ctypes.POINTER(ctypes.c_int64),
        ctypes.c_size_t,
    ]
    lib.axon_start_nrt_profile.restype = ctypes.c_int64
    lib.axon_stop_nrt_profile.argtypes = [ctypes.c_char_p]
    lib.axon_stop_nrt_profile.restype = ctypes.c_int64

    @contextlib.contextmanager
    def _hook(output_dir, device_ids):
        import jax

        jax.devices()
        if device_ids:
            ids = (ctypes.c_int64 * len(device_ids))(*device_ids)
            rc = lib.axon_start_nrt_profile(ids, len(device_ids))
        else:
            rc = lib.axon_start_nrt_profile(None, 0)
        if rc != 0:
            raise RuntimeError(f"axon_start_nrt_profile rc={rc}")
        try:
            yield
        finally:
            n = lib.axon_stop_nrt_profile(str(output_dir).encode())
            print(f"ntff profile: {n} file(s) written to {output_dir}")

    mod = types.ModuleType("antenv.axon_hooks")
    mod.get_axon_ntff_profile_hook = lambda: _hook
    mod.set_axon_ntff_profile_hook = lambda h: None
    sys.modules["antenv.axon_hooks"] = mod


def _run(x, weight, trace=False):
    from concourse.bass_utils import run_bass_kernel_spmd

    if trace:
        _install_axon_ntff_hook()
    nc = _get_nc()
    in_maps = _prep_inputs(x, weight)
    res = run_bass_kernel_spmd(
        nc, in_maps, core_ids=list(range(N_CORES)), trace=trace
    )
    return _unpack_output(res), res


def kernel(x, weight):
    out, _ = _run(x, weight, trace=False)
    return out


# revision 62
# speedup vs baseline: 1.1863x; 1.1863x over previous
# Binary linear: y[b,s,o] = sum_i x[b,s,i] * sign(W)[o,i]
#
# Strategy (8 NeuronCores, data-parallel over tokens):
#   - Host: flatten x to [32768, 768] and shard 8 x [4096, 768]. Per core,
#     pack x per 512-token group with the contraction dim on SBUF
#     partitions, p-major so every DMA lands with multi-KB contiguous
#     partition rows (small rows halve the DMA queues' effective rate).
#     Contraction blocks k0/k1 are quantized to fp8 e4m3, k2..k5 stay bf16.
#     Weights are sign(W) (exactly +-1): fp8 for k0/k1, bf16 for k2..k5.
#   - Device (per core): out[o-block, token] layout. Per (group, out-slab):
#     four bf16 matmuls (k2..k5, N=512) plus ONE fp8 DoubleRow matmul that
#     contracts k0+k1 together at ~1.44x the bf16 rate. Out-slabs run in
#     PAIRS with the k-loop interleaved between the two PSUM banks: a
#     single bank caps the accumulate stream at ~2.0GHz, alternating banks
#     sustains the full ~2.4GHz PE rate. Evictions are DVE f32->bf16 casts
#     (kept off the scalar engine, whose DMA issues would delay them and
#     stall PSUM recycling); y stores are linear 128KB DMAs balanced across
#     the two hardware DMA queues. A short PE warmup covers the ~3us DMA
#     launch+ramp latency.
#   - Accuracy: only x carries rounding error (w is exact): fp8 on 2/6 of
#     the contraction + bf16 elsewhere + bf16 y => rel err ~1.55e-2
#     (measured), within the 2e-2 gate with margin.
#   - Host: unpack [os][g][128, 512] -> [4, 8192, 768] f32.

import numpy as np

N_CORES = 8
B, S, D_IN, D_OUT = 4, 8192, 768, 768
T_TOTAL = B * S              # 32768 tokens
T_CORE = T_TOTAL // N_CORES  # 4096 tokens per core
P = 128
KB = D_IN // P               # 6 contraction blocks (k0/k1 fp8, k2-5 bf16)
OS = D_OUT // P              # 6 out-feature slabs
TG = 512                     # tokens per group (one PSUM bank of f32)
G = T_CORE // TG             # 8 groups per core
N_WARMUP = 9

_cache = {}


def _build():
    import concourse.bacc as bacc
    import concourse.mybir as mybir
    import concourse.tile as tile

    f32 = mybir.dt.float32
    bf16 = mybir.dt.bfloat16
    fp8 = mybir.dt.float8e4
    DR = mybir.MatmulPerfMode.DoubleRowSwInterleave

    nc = bacc.Bacc(
        "TRN2",
        target_bir_lowering=False,
        debug=False,
        num_devices=N_CORES,
    )

    # group 0 (1/8 of tokens) is computed FULLY in fp8 (3 DoubleRows instead
    # of 1 DR + 4 bf16 matmuls): saves ~2.5us of PE stream and shrinks the
    # critical head bytes; total rel err 1.73e-2 (measured) vs the 2e-2 gate
    x8P = nc.dram_tensor("x8P", [G, P, 2, TG], fp8, kind="ExternalInput")
    x8g0P = nc.dram_tensor("x8g0P", [2, P, 2, TG], fp8, kind="ExternalInput")
    xBP = nc.dram_tensor("xBP", [G, P, KB - 2, TG], bf16, kind="ExternalInput")
    w8P = nc.dram_tensor("w8P", [3, P, OS, 2, P], fp8, kind="ExternalInput")
    wBP = nc.dram_tensor("wBP", [OS, P, KB - 2, P], bf16, kind="ExternalInput")
    yP = nc.dram_tensor("yP", [OS, G, P, TG], bf16, kind="ExternalOutput")

    with tile.TileContext(nc) as tc:
        with (
            tc.tile_pool(name="wpool", bufs=1) as w_pool,
            tc.tile_pool(name="xpool", bufs=1) as x_pool,
            tc.tile_pool(name="ypool", bufs=14) as y_pool,
            tc.tile_pool(name="psum", bufs=7, space="PSUM") as psum_pool,
        ):
            # --- PE warmup: dummy matmuls on zeroed scratch so the PE clock
            # has ramped by the time the first real operands land. ---
            wu = x_pool.tile([P, P + TG], bf16, tag="wu", name="wu", bufs=1)
            nc.gpsimd.memset(wu[:], 0.0)
            wups = psum_pool.tile([P, TG], f32, tag="wups", name="wups", bufs=1)
            for _ in range(N_WARMUP):
                nc.tensor.matmul(
                    wups[:], wu[:, :P], wu[:, P:],
                    start=True, stop=True, skip_group_check=True,
                )
            wu_out = x_pool.tile([P, TG], bf16, tag="wuo", name="wuo", bufs=1)
            nc.vector.tensor_copy(wu_out[:], wups[:])

            # --- loads, interleaved across the two HW queues in need order
            # (group 0 consumes only fp8 operands; for g>=1 the k-order is
            # bf16 k2..k5 first, fp8 DR last, so the fp8 operands may
            # arrive later than the bf16 ones)
            w8k = [
                w_pool.tile([P, OS, 2, P], fp8, tag=f"w8_{kp}", name=f"w8_{kp}")
                for kp in range(3)
            ]
            x8g0kp = [
                x_pool.tile([P, 2, TG], fp8, tag=f"x8g0_{kp}", name=f"x8g0_{kp}")
                for kp in range(2)
            ]
            wB = [None] * OS

            def wB_load(os_, eng):
                t = w_pool.tile([P, KB - 2, P], bf16, tag=f"wB{os_}", name=f"wB{os_}")
                eng.dma_start(t[:], wBP[os_])
                wB[os_] = t

            x8 = [None] * G

            def x8_load(g, eng):
                t = x_pool.tile([P, 2, TG], fp8, tag=f"x8_{g}", name=f"x8_{g}")
                eng.dma_start(t[:], x8P[g])
                x8[g] = t

            xB = [None] * G
            xB1h = [None, None]

            def xB1_load(h, eng):
                # group 1 bf16 x as two k-pair tiles (2KB contiguous rows)
                # so both queues fill it in parallel just ahead of the PE
                t = x_pool.tile([P, 2, TG], bf16, tag=f"xB1_{h}", name=f"xB1_{h}")
                eng.dma_start(t[:], xBP[1, :, 2 * h : 2 * h + 2, :])
                xB1h[h] = t

            def xB_load(g, eng):
                t = x_pool.tile([P, KB - 2, TG], bf16, tag=f"xB{g}", name=f"xB{g}")
                eng.dma_start(t[:], xBP[g])
                xB[g] = t

            x8_load(0, nc.sync)
            nc.scalar.dma_start(w8k[0][:], w8P[0])
            nc.sync.dma_start(w8k[1][:], w8P[1])
            nc.scalar.dma_start(x8g0kp[0][:], x8g0P[0])
            nc.sync.dma_start(x8g0kp[1][:], x8g0P[1])
            nc.scalar.dma_start(w8k[2][:], w8P[2])
            wB_load(0, nc.sync)
            wB_load(1, nc.scalar)
            x8_load(1, nc.scalar)
            xB1_load(0, nc.sync)
            xB1_load(1, nc.scalar)
            wB_load(2, nc.sync)
            wB_load(3, nc.scalar)
            wB_load(4, nc.scalar)
            wB_load(5, nc.scalar)
            xB_load(3, nc.sync)
            xB_load(2, nc.scalar)
            x8_load(2, nc.scalar)
            x8_load(3, nc.scalar)
            xB_load(5, nc.sync)
            xB_load(4, nc.scalar)
            x8_load(4, nc.scalar)
            x8_load(5, nc.scalar)
            xB_load(6, nc.sync)
            x8_load(6, nc.scalar)
            x8_load(7, nc.scalar)
            xB_load(7, nc.sync)

            def rhsB(g, k):
                if g == 1:
                    return xB1h[(k - 2) // 2][:, (k - 2) % 2, :]
                return xB[g][:, k - 2, :]

            # --- main loop: out-slab pairs, k-loop interleaved across the
            # pair's two PSUM banks to sustain the full PE rate; bf16 k2..k5
            # first, then one fp8 DoubleRow matmul contracting k0+k1 ---
            ecnt = 0
            for g in range(G):
                for osp in range(OS // 2):
                    os_a, os_b = 2 * osp, 2 * osp + 1
                    final = g == G - 1 and osp == OS // 2 - 1
                    ps_a = psum_pool.tile([P, TG], f32, tag="ps", name=f"ps{g}_{os_a}")
                    ps_b = psum_pool.tile([P, TG], f32, tag="ps", name=f"ps{g}_{os_b}")
                    if g == 0:
                        # group 0: fully fp8 - three DoubleRow pairs
                        for kp in range(3):
                            r8 = x8[0][:] if kp == 0 else x8g0kp[kp - 1][:]
                            st, sp = kp == 0, kp == 2
                            nc.tensor.matmul(
                                ps_a[:], w8k[kp][:, os_a, :, :], r8,
                                start=st, stop=sp, perf_mode=DR,
                            )
                            nc.tensor.matmul(
                                ps_b[:], w8k[kp][:, os_b, :, :], r8,
                                start=st, stop=sp, perf_mode=DR,
                            )
                    else:
                        for k in range(2, KB):
                            st = k == 2
                            nc.tensor.matmul(
                                ps_a[:], wB[os_a][:, k - 2, :], rhsB(g, k),
                                start=st, stop=False, skip_group_check=final,
                            )
                            nc.tensor.matmul(
                                ps_b[:], wB[os_b][:, k - 2, :], rhsB(g, k),
                                start=st, stop=False, skip_group_check=final,
                            )
                    if final:
                        # tail pair: split the closing DR matmuls into
                        # column halves so the left half's eviction+store
                        # overlaps the right half's compute, and fan the
                        # four lanes over both copy engines and DMA queues
                        h = TG // 2
                        yt_a = y_pool.tile([P, TG], bf16, tag="y", name="yfa")
                        yt_b = y_pool.tile([P, TG], bf16, tag="y", name="yfb")
                        # all four half-DRs first (emitting evictions in
                        # between makes Tile serialize the later DRs behind
                        # the earlier reads), then the four evict+store
                        # lanes - the left lanes' sems only depend on the
                        # left DRs, so they overlap the right DRs' compute
                        for lo, hi in ((0, h), (h, TG)):
                            r8 = x8[g][:, :, lo:hi]
                            nc.tensor.matmul(
                                ps_a[:, lo:hi], w8k[0][:, os_a, :, :], r8,
                                start=False, stop=hi == TG, perf_mode=DR,
                                skip_group_check=True,
                            )
                            nc.tensor.matmul(
                                ps_b[:, lo:hi], w8k[0][:, os_b, :, :], r8,
                                start=False, stop=hi == TG, perf_mode=DR,
                                skip_group_check=True,
                            )
                        for lo, hi in ((0, h), (h, TG)):
                            nc.vector.tensor_copy(
                                yt_a[:, lo:hi], ps_a[:, lo:hi]
                            )
                            nc.scalar.copy(yt_b[:, lo:hi], ps_b[:, lo:hi])
                            nc.sync.dma_start(
                                yP[os_a, g, :, lo:hi], yt_a[:, lo:hi]
                            )
                            nc.scalar.dma_start(
                                yP[os_b, g, :, lo:hi], yt_b[:, lo:hi]
                            )
                        continue
                    if g != 0:
                        rhs8 = x8[g][:]
                        nc.tensor.matmul(
                            ps_a[:], w8k[0][:, os_a, :, :], rhs8,
                            start=False, stop=True, perf_mode=DR,
                        )
                        nc.tensor.matmul(
                            ps_b[:], w8k[0][:, os_b, :, :], rhs8,
                            start=False, stop=True, perf_mode=DR,
                        )
                    for os_, ps in ((os_a, ps_a), (os_b, ps_b)):
                        yt = y_pool.tile([P, TG], bf16, tag="y", name=f"y{g}_{os_}")
                        # all evictions on DVE: the scalar engine's DMA
                        # issues would delay them and stall PSUM recycling
                        nc.vector.tensor_copy(yt[:], ps[:])
                        # store queues: scalar while sync still streams x,
                        # alternating afterwards so neither queue backs up
                        # near the tail
                        if g <= 4:
                            q = nc.scalar
                        else:
                            q = nc.sync if os_ % 2 == 0 else nc.scalar
                        q.dma_start(yP[os_, g], yt[:])
                        ecnt += 1

    nc.compile()
    return nc


def _get_nc():
    if "nc" not in _cache:
        _cache["nc"] = _build()
    return _cache["nc"]


def _prep_inputs(x, weight):
    import ml_dtypes

    x = np.asarray(x, dtype=np.float32)
    w = np.asarray(weight, dtype=np.float32)
    x2 = x.reshape(N_CORES, T_CORE, D_IN)
    # x5[c, g, t, k, p] = x2[c, g*TG + t, k*P + p] -> packs [c, g, p, k, t]
    x5 = x2.reshape(N_CORES, G, TG, KB, P)
    x8Pack = np.ascontiguousarray(x5[:, :, :, :2, :].transpose(0, 1, 4, 3, 2)).astype(
        ml_dtypes.float8_e4m3fn
    )
    xBPack = np.ascontiguousarray(x5[:, :, :, 2:, :].transpose(0, 1, 4, 3, 2)).astype(
        ml_dtypes.bfloat16
    )
    # group-0 k2..k5 x in fp8 as two pair-major k-pair tiles [c, kp, p, 2, t]
    x8g0Pack = np.ascontiguousarray(
        x5[:, 0, :, 2:, :].transpose(0, 3, 2, 1)          # [c, p, k4, t]
        .reshape(N_CORES, P, 2, 2, TG).transpose(0, 2, 1, 3, 4)
    ).astype(ml_dtypes.float8_e4m3fn)
    # S4[os, o, k, p] = sign(W)[os*P + o, k*P + p]  (+-1/0 exact in both)
    S4 = np.sign(w).reshape(OS, P, KB, P)
    # SwInterleave weight layout, one pack per k-pair: per partition row the
    # pair's weights are interleaved pairwise with output columns reversed:
    # [A127, B127, A126, B126, ..., A0, B0]  (A=2kp, B=2kp+1)
    w8Pack = np.ascontiguousarray(
        np.stack(
            [
                S4[:, ::-1, 2 * kp : 2 * kp + 2, :]
                .transpose(3, 0, 1, 2)
                .reshape(P, OS, 2, P)
                for kp in range(3)
            ]
        )
    ).astype(ml_dtypes.float8_e4m3fn)
    wBPack = np.ascontiguousarray(S4[:, :, 2:, :].transpose(0, 3, 2, 1)).astype(
        ml_dtypes.bfloat16
    )
    return [
        {
            "x8P": x8Pack[c],
            "x8g0P": x8g0Pack[c],
            "xBP": xBPack[c],
            "w8P": w8Pack,
            "wBP": wBPack,
        }
        for c in range(N_CORES)
    ]


def _unpack_output(res):
    # yP [OS, G, P(o), TG(t)] -> y_core [T_CORE, D_OUT]
    outs = []
    for r in res.results:
        yp = np.asarray(r["yP"]).astype(np.float32)
        outs.append(yp.transpose(1, 3, 0, 2).reshape(T_CORE, D_OUT))
    return np.concatenate(outs, axis=0).reshape(B, S, D_OUT)


def _install_axon_ntff_hook():
    """The agent image's `antenv` lacks `axon_hooks`; register an equivalent
    module backed by direct ctypes calls into libaxon_pjrt.so so that
    run_bass_kernel_spmd(trace=True) can capture NTFF profiles under axon."""
    import sys

    if "antenv.axon_hooks" in sys.modules:
        return
    import contextlib
    import ctypes
    import types

    so_path = "/opt/axon/libaxon_pjrt.so"
    try:
        lib = ctypes.CDLL(so_path)
    except OSError:
        return
    if not hasattr(lib, "axon_start_nrt_profile"):
        return
    lib.axon_start_nrt_profile.argtypes = [
        ctypes.POINTER(ctypes.c_int64),
        ctypes.c_size_t,
    ]
    lib.axon_start_nrt_profile.restype = ctypes.c_int64
    lib.axon_stop_nrt_profile.argtypes = [ctypes.c_char_p]
    lib.axon_stop_nrt_profile.restype = ctypes.c_int64

    @contextlib.contextmanager
    def _hook(output_dir, device_ids):
        import jax

        jax.devices()
        if device_ids:
            ids = (ctypes.c_int64 * len(device_ids))(*device_ids)
            rc = lib.axon_start_nrt_profile(ids, len(device_ids))
        else:
            rc = lib.axon_start_nrt_profile(None, 0)
        if rc != 0:
            raise RuntimeError(f"axon_start_nrt_profile rc={rc}")
        try:
            yield
        finally:
            n = lib.axon_stop_nrt_profile(str(output_dir).encode())
            print(f"ntff profile: {n} file(s) written to {output_dir}")

    mod = types.ModuleType("antenv.axon_hooks")
    mod.get_axon_ntff_profile_hook = lambda: _hook
    mod.set_axon_ntff_profile_hook = lambda h: None
    sys.modules["antenv.axon_hooks"] = mod


def _run(x, weight, trace=False):
    from concourse.bass_utils import run_bass_kernel_spmd

    if trace:
        _install_axon_ntff_hook()
    nc = _get_nc()
    in_maps = _prep_inputs(x, weight)
    res = run_bass_kernel_spmd(
        nc, in_maps, core_ids=list(range(N_CORES)), trace=trace
    )
    return _unpack_output(res), res


def kernel(x, weight):
    out, _ = _run(x, weight, trace=False)
    return out
